# revision 10
# baseline (speedup 1.0000x reference)
"""MFA per-component log-likelihood kernel for 8x TRN2 NeuronCores.

Math: out[n,k] = base[k] + sum_m (x_n . g_km)^2 + x_n . Wx_k + (x_n^2) . Wxx_k
with g/Wx/base from the Woodbury factorization (host-side, tiny).

Device strategy (per core, N_SHARD=2048 rows, output TRANSPOSED [K, n]):
  - Weights-stationary fp8 (e4m3) DoubleRow matmuls for the factor ("quad")
    columns: stationary = Gw block [128d, 2, 128cols], moving = x_fp8
    [128d, 2, 512n] -> psum y-block [128cols, 512n], 256-deep contraction
    per streamed column (2x fp32 MAC rate).
  - ScalarE squares psum with scale 1/64 into fp8 "sq" pairs.
  - Group-of-16 reduction on the PE: fp8 DoubleRow matmul against a
    constant 0/4 block-indicator matrix (S2), accumulating quad directly
    into the per-n-block accumulator psum bank [128k, 512n].
  - Linear term x.Wx: fp8 DoubleRow, stationary Wx block, same moving x,
    accumulated into the same acc bank.
  - x^2 . Wxx: fp16 matmuls (host-computed xsq fp16) into the same bank.
  - DVE adds base (per-partition scalar) while copying acc psum -> SBUF,
    DMA out.  Host transposes [K,N] -> [N,K].

Sharding: rows N=16384 split across 8 cores; params replicated.
"""

import math

import numpy as np

K, D_FEAT, L_FAC, N = 128, 1024, 16, 16384
N_CORES = 8
N_SHARD = N // N_CORES            # 2048 rows per core
NB = N_SHARD // 512               # 4 moving blocks of 512 rows
J2 = 4                            # DoubleRow contraction chunks (256 each)
JJ = 8                            # psb fp16 chunks
CB = 16                           # 128-col blocks of factor columns
NPAIR = CB // 2                   # S2 pair matmuls per n-block
GCOLS = K * L_FAC                 # 2048 factor columns
WCOLS = GCOLS + K                 # 2176 = [Gw | Wx]
SG = 32.0                         # Gw fp8 scale
SQ_SCALE = 1.0 / 64.0             # scalar: sq = (psum/64)^2 = y^2/4
S2_VAL = 4.0                      # un-scales sq in the group-sum matmul

_CACHE = {}


def _get_nc():
    if "nc" in _CACHE:
        return _CACHE["nc"]

    import concourse.bass as bass
    import concourse.tile as tile
    from concourse import bacc, mybir

    f32 = mybir.dt.float32
    f16 = mybir.dt.float16
    f8 = mybir.dt.float8e4
    DR = mybir.MatmulPerfMode.DoubleRow
    nc = bacc.Bacc("TRN2", target_bir_lowering=False, debug=False,
                   num_devices=N_CORES)

    xq = nc.dram_tensor("xq", [128, J2, 2, N_SHARD], f8, kind="ExternalInput").ap()
    xsqh = nc.dram_tensor("xsqh", [128, JJ, N_SHARD], f16, kind="ExternalInput").ap()
    wq = nc.dram_tensor("wq", [128, J2, 2, WCOLS], f8, kind="ExternalInput").ap()
    wxxh = nc.dram_tensor("wxxh", [128, JJ, K], f16, kind="ExternalInput").ap()
    s2 = nc.dram_tensor("s2", [128, NPAIR, 2, K], f8, kind="ExternalInput").ap()
    bs = nc.dram_tensor("bs", [128, 1], f32, kind="ExternalInput").ap()
    outT = nc.dram_tensor("outT", [128, N_SHARD], f32, kind="ExternalOutput").ap()

    WPIECES = [(0, 512), (512, 1024), (1024, 1536), (1536, WCOLS)]

    with tile.TileContext(nc) as tc:
        with (
            tc.tile_pool(name="singles", bufs=1) as singles,
            tc.tile_pool(name="sqpool", bufs=3) as sqpool,
            tc.tile_pool(name="upool", bufs=2) as upool,
            tc.tile_pool(name="qp", bufs=4, space="PSUM") as qp,
            tc.tile_pool(name="accp", bufs=4, space="PSUM") as accp,
        ):
            wq_s = singles.tile([128, J2, 2, WCOLS], f8, tag="wq")
            xq_s = singles.tile([128, J2, 2, N_SHARD], f8, tag="xq")
            xs_s = singles.tile([128, JJ, N_SHARD], f16, tag="xs")
            wxx_s = singles.tile([128, JJ, K], f16, tag="wxx")
            s2_s = singles.tile([128, NPAIR, 2, K], f8, tag="s2")
            bs_s = singles.tile([128, 1], f32, tag="bs")
            dmy = singles.tile([128, 2, 512], f8, tag="dmy")

            # Dummy matmuls on a memset tile warm the PE p-state while the
            # first weight/x DMAs are in flight.  The memset goes on the
            # otherwise-idle vector queue so it runs immediately.
            nc.vector.memset(dmy, 0)

            # DMA triggers spread across queues; issue order per queue is the
            # prefetch schedule (earliest-needed first).  Scalar's queue is
            # kept free for the squares.
            nc.sync.dma_start(out=wq_s[:, :, :, 0:512], in_=wq[:, :, :, 0:512])
            nc.sync.dma_start(out=xq_s[:, :, :, 0:512], in_=xq[:, :, :, 0:512])
            nc.sync.dma_start(out=s2_s, in_=s2)
            nc.sync.dma_start(out=wxx_s, in_=wxxh)
            nc.sync.dma_start(out=bs_s, in_=bs)
            for lo, hi in WPIECES[1:]:
                nc.sync.dma_start(out=wq_s[:, :, :, lo:hi], in_=wq[:, :, :, lo:hi])
            for nb in range(1, NB):
                nc.gpsimd.dma_start(out=xq_s[:, :, :, nb * 512:(nb + 1) * 512],
                                    in_=xq[:, :, :, nb * 512:(nb + 1) * 512])
            for nb in range(NB):
                nc.gpsimd.dma_start(out=xs_s[:, :, nb * 512:(nb + 1) * 512],
                                    in_=xsqh[:, :, nb * 512:(nb + 1) * 512])

            for _ in range(8):
                wrm = qp.tile([128, 512], f32, tag="q")
                nc.tensor.matmul(wrm, dmy[:, :, 0:128], dmy,
                                 start=True, stop=True, perf_mode=DR)

            # A matmul (its LDWEIGHTS) can wait on at most one not-yet-observed
            # semaphore, so touch each weight tensor with a throwaway matmul
            # before its first real use; real matmuls then wait only on their
            # data-side producer.
            for j2 in range(J2):
                warm = qp.tile([128, 512], f32, tag="q")
                nc.tensor.matmul(warm, wq_s[:, j2, :, 0:128],
                                 wq_s[:, j2, :, 0:512],
                                 start=True, stop=True, perf_mode=DR)

            accs = []

            def emit_psb(nb, open_, close):
                """psb matmuls into acc[nb]; open_/close start/end the psum
                accumulation group."""
                acc = accs[nb]
                nbs = slice(nb * 512, (nb + 1) * 512)
                for jj in range(JJ):
                    nc.tensor.matmul(acc, wxx_s[:, jj], xs_s[:, jj, nbs],
                                     start=(open_ and jj == 0),
                                     stop=(close and jj == JJ - 1))

            def emit_out(nb):
                u = upool.tile([128, 512], f32, tag="u")
                nc.vector.tensor_scalar_add(out=u, in0=accs[nb], scalar1=bs_s)
                nc.gpsimd.dma_start(
                    out=outT[:, nb * 512:(nb + 1) * 512], in_=u)

            for nb in range(NB):
                acc = accp.tile([128, 512], f32, tag="acc")
                accs.append(acc)
                nbs = slice(nb * 512, (nb + 1) * 512)
                if nb >= 1:
                    # psb for this block up front (xsq landed long ago), and
                    # the deferred psb+close for block 0 at the start of
                    # section 1.
                    if nb == 1:
                        emit_psb(0, open_=False, close=True)
                        emit_out(0)
                    emit_psb(nb, open_=True, close=False)
                pending = None
                for pair in range(NPAIR):
                    sq_t = sqpool.tile([128, 2, 512], f8, tag="sq")
                    for r in range(2):
                        cb = 2 * pair + r
                        q = qp.tile([128, 512], f32, tag="q")
                        for j2 in range(J2):
                            nc.tensor.matmul(
                                q, wq_s[:, j2, :, cb * 128:(cb + 1) * 128],
                                xq_s[:, j2, :, nbs],
                                start=(j2 == 0), stop=(j2 == J2 - 1),
                                perf_mode=DR)
                        nc.scalar.activation(
                            sq_t[:, r, :], q,
                            mybir.ActivationFunctionType.Square,
                            scale=SQ_SCALE)
                    if nb == 0 and pair == 0:
                        # warm s2 / wxxh before their first real matmuls
                        warm2 = qp.tile([128, 512], f32, tag="q")
                        nc.tensor.matmul(warm2[:, 0:128], s2_s[:, 0], s2_s[:, 0],
                                         start=True, stop=True, perf_mode=DR)
                        warm3 = qp.tile([128, 512], f32, tag="q")
                        nc.tensor.matmul(warm3[:, 0:128], wxx_s[:, 0],
                                         wxx_s[:, 0], start=True, stop=True)
                    # defer the group-sum one pair so the square can finish
                    if pending is not None:
                        p_pair, p_sq = pending
                        nc.tensor.matmul(acc, s2_s[:, p_pair], p_sq,
                                         start=(p_pair == 0 and nb == 0),
                                         stop=False, perf_mode=DR)
                    pending = (pair, sq_t)
                p_pair, p_sq = pending
                nc.tensor.matmul(acc, s2_s[:, p_pair], p_sq,
                                 start=False, stop=False, perf_mode=DR)
                # linear term x . Wx (fp8, same moving x); closes the group
                # for nb >= 1 (nb 0 closes via its deferred psb).
                for j2 in range(J2):
                    nc.tensor.matmul(acc, wq_s[:, j2, :, GCOLS:WCOLS],
                                     xq_s[:, j2, :, nbs],
                                     start=False,
                                     stop=(nb >= 1 and j2 == J2 - 1),
                                     perf_mode=DR)
                if nb >= 1:
                    emit_out(nb)

    nc.finalize()
    _CACHE["nc"] = nc
    return nc


def _host_params(PI, MU, A, D):
    import ml_dtypes
    FP8 = ml_dtypes.float8_e4m3

    PI64 = PI.astype(np.float64)
    MU64 = MU.astype(np.float64)
    A64 = A.astype(np.float64)
    D64 = D.astype(np.float64)

    iD = D64 ** -2.0                                   # (K, d)
    iDA = iD[:, :, None] * A64                         # (K, d, l)
    Lm = np.eye(L_FAC)[None] + np.einsum("kdl,kdm->klm", A64, iDA)
    iL = np.linalg.inv(Lm)
    C = np.linalg.cholesky(iL)                         # iL = C C^T
    s = 1.0 / math.sqrt(2.0)
    G = np.einsum("kdl,klm->kdm", iDA, C) * s          # (K, d, l)
    b = np.einsum("kd,kdl->kl", MU64, iDA)             # (K, l)
    h = np.einsum("kl,klm->km", b, C) * s              # (K, l)

    Gw = G.transpose(1, 0, 2).reshape(D_FEAT, GCOLS)   # col k*16+m
    Wx = (iD * MU64).T - 2.0 * np.einsum("kdm,km->kd", G, h).T
    Wxx = -0.5 * iD.T                                  # (d, K)

    det_L = np.linalg.slogdet(Lm)[1]
    log_det_sigma = det_L - np.sum(np.log(iD), axis=1)
    c1 = np.sum(iD * MU64 * MU64, axis=1)
    hsq = np.sum(h * h, axis=1)
    base = PI64 - 0.5 * (D_FEAT * math.log(2.0 * math.pi)
                         + log_det_sigma + c1) + hsq

    wcat = np.concatenate([Gw * SG, Wx], axis=1).astype(np.float32)  # (d, 2176)
    wq = np.ascontiguousarray(
        wcat.astype(FP8).reshape(J2, 2, 128, WCOLS).transpose(2, 0, 1, 3))

    wxxh = np.ascontiguousarray(
        Wxx.astype(np.float32).reshape(JJ, 128, K).transpose(1, 0, 2)
    ).astype(np.float16)

    s2 = np.zeros((128, NPAIR, 2, K), dtype=np.float32)
    p_idx = np.arange(128)
    for pair in range(NPAIR):
        for r in range(2):
            cb = 2 * pair + r
            s2[p_idx, pair, r, cb * 8 + p_idx // 16] = S2_VAL
    s2 = s2.astype(FP8)

    bs = np.ascontiguousarray(base.astype(np.float32).reshape(128, 1))
    return wq, wxxh, s2, bs


def kernel(x, PI, MU, A, D, _trace=False):
    from concourse.bass_utils import run_bass_kernel_spmd
    import ml_dtypes
    FP8 = ml_dtypes.float8_e4m3

    x = np.asarray(x, dtype=np.float32)
    wq, wxxh, s2, bs = _host_params(
        np.asarray(PI), np.asarray(MU), np.asarray(A), np.asarray(D))

    in_maps = []
    for c in range(N_CORES):
        xs = x[c * N_SHARD:(c + 1) * N_SHARD]          # (2048, 1024)
        x8t = xs.astype(FP8).T                         # (1024, 2048)
        xq = np.ascontiguousarray(
            x8t.reshape(J2, 2, 128, N_SHARD).transpose(2, 0, 1, 3))
        xsqt = (xs * xs).astype(np.float16).T          # (1024, 2048)
        xsqh = np.ascontiguousarray(
            xsqt.reshape(JJ, 128, N_SHARD).transpose(1, 0, 2))
        in_maps.append({
            "xq": xq,
            "xsqh": xsqh,
            "wq": wq,
            "wxxh": wxxh,
            "s2": s2,
            "bs": bs,
        })

    nc = _get_nc()
    res = run_bass_kernel_spmd(nc, in_maps, list(range(N_CORES)),
                               trace=_trace)
    _CACHE["last_results"] = res
    outT = np.concatenate([res.results[c]["outT"] for c in range(N_CORES)],
                          axis=1)                      # (128, 16384)
    return np.ascontiguousarray(outT.T).astype(np.float32)


# revision 11
# speedup vs baseline: 1.0246x; 1.0246x over previous
"""MFA per-component log-likelihood kernel for 8x TRN2 NeuronCores.

Math: out[n,k] = base[k] + sum_m (x_n . g_km)^2 + x_n . Wx_k + (x_n^2) . Wxx_k
with g/Wx/base from the Woodbury factorization (host-side, tiny).

Device strategy (per core, N_SHARD=2048 rows, output TRANSPOSED [K, n]):
  - Weights-stationary fp8 (e4m3) DoubleRow matmuls for the factor ("quad")
    columns: stationary = Gw block [128d, 2, 128cols], moving = x_fp8
    [128d, 2, 512n] -> psum y-block [128cols, 512n], 256-deep contraction
    per streamed column (2x fp32 MAC rate).
  - ScalarE squares psum with scale 1/64 into fp8 "sq" pairs.
  - Group-of-16 reduction on the PE: fp8 DoubleRow matmul against a
    constant 0/4 block-indicator matrix (S2), accumulating quad directly
    into the per-n-block accumulator psum bank [128k, 512n].
  - Linear term x.Wx: fp8 DoubleRow, stationary Wx block, same moving x,
    accumulated into the same acc bank.
  - x^2 . Wxx: fp16 matmuls (host-computed xsq fp16) into the same bank.
  - DVE adds base (per-partition scalar) while copying acc psum -> SBUF,
    DMA out.  Host transposes [K,N] -> [N,K].

Sharding: rows N=16384 split across 8 cores; params replicated.
"""

import math

import numpy as np

K, D_FEAT, L_FAC, N = 128, 1024, 16, 16384
N_CORES = 8
N_SHARD = N // N_CORES            # 2048 rows per core
NB = N_SHARD // 512               # 4 moving blocks of 512 rows
J2 = 4                            # DoubleRow contraction chunks (256 each)
JJ = 8                            # psb fp16 chunks
CB = 16                           # 128-col blocks of factor columns
NPAIR = CB // 2                   # S2 pair matmuls per n-block
GCOLS = K * L_FAC                 # 2048 factor columns
WCOLS = GCOLS + K                 # 2176 = [Gw | Wx]
SG = 32.0                         # Gw fp8 scale
SQ_SCALE = 1.0 / 64.0             # scalar: sq = (psum/64)^2 = y^2/4
S2_VAL = 4.0                      # un-scales sq in the group-sum matmul

_CACHE = {}


def _get_nc():
    if "nc" in _CACHE:
        return _CACHE["nc"]

    import concourse.bass as bass
    import concourse.tile as tile
    from concourse import bacc, mybir

    f32 = mybir.dt.float32
    f16 = mybir.dt.float16
    f8 = mybir.dt.float8e4
    DR = mybir.MatmulPerfMode.DoubleRow
    nc = bacc.Bacc("TRN2", target_bir_lowering=False, debug=False,
                   num_devices=N_CORES)

    xq = nc.dram_tensor("xq", [128, J2, 2, N_SHARD], f8, kind="ExternalInput").ap()
    xsqh = nc.dram_tensor("xsqh", [128, JJ, N_SHARD], f16, kind="ExternalInput").ap()
    wq = nc.dram_tensor("wq", [128, J2, 2, WCOLS], f8, kind="ExternalInput").ap()
    wxxh = nc.dram_tensor("wxxh", [128, JJ, K], f16, kind="ExternalInput").ap()
    s2 = nc.dram_tensor("s2", [128, NPAIR, 2, K], f8, kind="ExternalInput").ap()
    bs = nc.dram_tensor("bs", [128, 1], f32, kind="ExternalInput").ap()
    outT = nc.dram_tensor("outT", [128, N_SHARD], f32, kind="ExternalOutput").ap()

    WPIECES = [(0, 512), (512, 1024), (1024, 1536), (1536, WCOLS)]

    with tile.TileContext(nc) as tc:
        with (
            tc.tile_pool(name="singles", bufs=1) as singles,
            tc.tile_pool(name="sqpool", bufs=3) as sqpool,
            tc.tile_pool(name="upool", bufs=2) as upool,
            tc.tile_pool(name="qp", bufs=4, space="PSUM") as qp,
            tc.tile_pool(name="accp", bufs=4, space="PSUM") as accp,
        ):
            wq_s = singles.tile([128, J2, 2, WCOLS], f8, tag="wq")
            xq_s = singles.tile([128, J2, 2, N_SHARD], f8, tag="xq")
            xs_s = singles.tile([128, JJ, N_SHARD], f16, tag="xs")
            wxx_s = singles.tile([128, JJ, K], f16, tag="wxx")
            s2_s = singles.tile([128, NPAIR, 2, K], f8, tag="s2")
            bs_s = singles.tile([128, 1], f32, tag="bs")
            dmy = singles.tile([128, 2, 512], f8, tag="dmy")

            # Dummy matmuls on a memset tile warm the PE p-state while the
            # first weight/x DMAs are in flight.  The memset goes on the
            # otherwise-idle vector queue so it runs immediately.
            nc.vector.memset(dmy, 0)

            # DMA triggers spread across queues; issue order per queue is the
            # prefetch schedule (earliest-needed first).  Scalar's queue is
            # kept free for the squares.
            nc.sync.dma_start(out=wq_s[:, :, :, 0:512], in_=wq[:, :, :, 0:512])
            nc.sync.dma_start(out=s2_s, in_=s2)
            nc.sync.dma_start(out=wxx_s, in_=wxxh)
            nc.sync.dma_start(out=xq_s[:, :, :, 0:512], in_=xq[:, :, :, 0:512])
            for lo, hi in WPIECES[1:]:
                nc.sync.dma_start(out=wq_s[:, :, :, lo:hi], in_=wq[:, :, :, lo:hi])
            nc.sync.dma_start(out=bs_s, in_=bs)
            nc.gpsimd.dma_start(out=xq_s[:, :, :, 512:1024],
                                in_=xq[:, :, :, 512:1024])
            nc.gpsimd.dma_start(out=xs_s[:, :, 0:512], in_=xsqh[:, :, 0:512])
            nc.gpsimd.dma_start(out=xs_s[:, :, 512:1024],
                                in_=xsqh[:, :, 512:1024])
            nc.gpsimd.dma_start(out=xq_s[:, :, :, 1024:1536],
                                in_=xq[:, :, :, 1024:1536])
            nc.gpsimd.dma_start(out=xs_s[:, :, 1024:1536],
                                in_=xsqh[:, :, 1024:1536])
            nc.gpsimd.dma_start(out=xq_s[:, :, :, 1536:2048],
                                in_=xq[:, :, :, 1536:2048])
            nc.gpsimd.dma_start(out=xs_s[:, :, 1536:2048],
                                in_=xsqh[:, :, 1536:2048])

            for _ in range(6):
                wrm = qp.tile([128, 512], f32, tag="q")
                nc.tensor.matmul(wrm, dmy[:, :, 0:128], dmy,
                                 start=True, stop=True, perf_mode=DR)

            # A matmul (its LDWEIGHTS) can wait on at most one not-yet-observed
            # semaphore, so touch each weight tensor with a throwaway matmul
            # before its first real use; real matmuls then wait only on their
            # data-side producer.
            for j2 in range(J2):
                warm = qp.tile([128, 512], f32, tag="q")
                nc.tensor.matmul(warm, wq_s[:, j2, :, 0:128],
                                 wq_s[:, j2, :, 0:512],
                                 start=True, stop=True, perf_mode=DR)

            accs = []

            def emit_psa(nb):
                """x . Wx into acc[nb]; opens the accumulation group."""
                acc = accs[nb]
                nbs = slice(nb * 512, (nb + 1) * 512)
                for j2 in range(J2):
                    nc.tensor.matmul(acc, wq_s[:, j2, :, GCOLS:WCOLS],
                                     xq_s[:, j2, :, nbs],
                                     start=(j2 == 0), stop=False,
                                     perf_mode=DR)

            def emit_psb(nb, close):
                """x^2 . Wxx into acc[nb]; close=True ends the group."""
                acc = accs[nb]
                nbs = slice(nb * 512, (nb + 1) * 512)
                for jj in range(JJ):
                    nc.tensor.matmul(acc, wxx_s[:, jj], xs_s[:, jj, nbs],
                                     start=False,
                                     stop=(close and jj == JJ - 1))

            def emit_out(nb):
                u = upool.tile([128, 512], f32, tag="u")
                nc.vector.tensor_scalar_add(out=u, in0=accs[nb], scalar1=bs_s)
                nc.gpsimd.dma_start(
                    out=outT[:, nb * 512:(nb + 1) * 512], in_=u)

            for nb in range(NB):
                acc = accp.tile([128, 512], f32, tag="acc")
                accs.append(acc)
                nbs = slice(nb * 512, (nb + 1) * 512)
                emit_psa(nb)
                pending = []
                for pair in range(NPAIR):
                    sq_t = sqpool.tile([128, 2, 512], f8, tag="sq")
                    for r in range(2):
                        cb = 2 * pair + r
                        q = qp.tile([128, 512], f32, tag="q")
                        for j2 in range(J2):
                            nc.tensor.matmul(
                                q, wq_s[:, j2, :, cb * 128:(cb + 1) * 128],
                                xq_s[:, j2, :, nbs],
                                start=(j2 == 0), stop=(j2 == J2 - 1),
                                perf_mode=DR)
                        nc.scalar.activation(
                            sq_t[:, r, :], q,
                            mybir.ActivationFunctionType.Square,
                            scale=SQ_SCALE)
                    if nb == 0 and pair == 1:
                        # warm s2 / wxxh before their first real matmuls
                        warm2 = qp.tile([128, 512], f32, tag="q")
                        nc.tensor.matmul(warm2[:, 0:128], s2_s[:, 0], s2_s[:, 0],
                                         start=True, stop=True, perf_mode=DR)
                        warm3 = qp.tile([128, 512], f32, tag="q")
                        nc.tensor.matmul(warm3[:, 0:128], wxx_s[:, 0],
                                         wxx_s[:, 0], start=True, stop=True)
                    if nb >= 1 and pair == 3:
                        # mid-section: close the previous block (its deferred
                        # psb) and start this block's psb -- xsq has landed.
                        if nb == 1:
                            emit_psb(0, close=True)
                            emit_out(0)
                        emit_psb(nb, close=False)
                    # group-sum deferred two pairs so squares stay off the
                    # critical path
                    pending.append((pair, sq_t))
                    if len(pending) > 2:
                        p_pair, p_sq = pending.pop(0)
                        nc.tensor.matmul(acc, s2_s[:, p_pair], p_sq,
                                         start=False, stop=False,
                                         perf_mode=DR)
                while pending:
                    p_pair, p_sq = pending.pop(0)
                    last = (not pending) and nb >= 1
                    nc.tensor.matmul(acc, s2_s[:, p_pair], p_sq,
                                     start=False, stop=last, perf_mode=DR)
                if nb >= 1:
                    emit_out(nb)

    nc.finalize()
    _CACHE["nc"] = nc
    return nc


def _host_params(PI, MU, A, D):
    import ml_dtypes
    FP8 = ml_dtypes.float8_e4m3

    PI64 = PI.astype(np.float64)
    MU64 = MU.astype(np.float64)
    A64 = A.astype(np.float64)
    D64 = D.astype(np.float64)

    iD = D64 ** -2.0                                   # (K, d)
    iDA = iD[:, :, None] * A64                         # (K, d, l)
    Lm = np.eye(L_FAC)[None] + np.einsum("kdl,kdm->klm", A64, iDA)
    iL = np.linalg.inv(Lm)
    C = np.linalg.cholesky(iL)                         # iL = C C^T
    s = 1.0 / math.sqrt(2.0)
    G = np.einsum("kdl,klm->kdm", iDA, C) * s          # (K, d, l)
    b = np.einsum("kd,kdl->kl", MU64, iDA)             # (K, l)
    h = np.einsum("kl,klm->km", b, C) * s              # (K, l)

    Gw = G.transpose(1, 0, 2).reshape(D_FEAT, GCOLS)   # col k*16+m
    Wx = (iD * MU64).T - 2.0 * np.einsum("kdm,km->kd", G, h).T
    Wxx = -0.5 * iD.T                                  # (d, K)

    det_L = np.linalg.slogdet(Lm)[1]
    log_det_sigma = det_L - np.sum(np.log(iD), axis=1)
    c1 = np.sum(iD * MU64 * MU64, axis=1)
    hsq = np.sum(h * h, axis=1)
    base = PI64 - 0.5 * (D_FEAT * math.log(2.0 * math.pi)
                         + log_det_sigma + c1) + hsq

    wcat = np.concatenate([Gw * SG, Wx], axis=1).astype(np.float32)  # (d, 2176)
    wq = np.ascontiguousarray(
        wcat.astype(FP8).reshape(J2, 2, 128, WCOLS).transpose(2, 0, 1, 3))

    wxxh = np.ascontiguousarray(
        Wxx.astype(np.float32).reshape(JJ, 128, K).transpose(1, 0, 2)
    ).astype(np.float16)

    s2 = np.zeros((128, NPAIR, 2, K), dtype=np.float32)
    p_idx = np.arange(128)
    for pair in range(NPAIR):
        for r in range(2):
            cb = 2 * pair + r
            s2[p_idx, pair, r, cb * 8 + p_idx // 16] = S2_VAL
    s2 = s2.astype(FP8)

    bs = np.ascontiguousarray(base.astype(np.float32).reshape(128, 1))
    return wq, wxxh, s2, bs


def kernel(x, PI, MU, A, D, _trace=False):
    from concourse.bass_utils import run_bass_kernel_spmd
    import ml_dtypes
    FP8 = ml_dtypes.float8_e4m3

    x = np.asarray(x, dtype=np.float32)
    wq, wxxh, s2, bs = _host_params(
        np.asarray(PI), np.asarray(MU), np.asarray(A), np.asarray(D))

    in_maps = []
    for c in range(N_CORES):
        xs = x[c * N_SHARD:(c + 1) * N_SHARD]          # (2048, 1024)
        x8t = xs.astype(FP8).T                         # (1024, 2048)
        xq = np.ascontiguousarray(
            x8t.reshape(J2, 2, 128, N_SHARD).transpose(2, 0, 1, 3))
        xsqt = (xs * xs).astype(np.float16).T          # (1024, 2048)
        xsqh = np.ascontiguousarray(
            xsqt.reshape(JJ, 128, N_SHARD).transpose(1, 0, 2))
        in_maps.append({
            "xq": xq,
            "xsqh": xsqh,
            "wq": wq,
            "wxxh": wxxh,
            "s2": s2,
            "bs": bs,
        })

    nc = _get_nc()
    res = run_bass_kernel_spmd(nc, in_maps, list(range(N_CORES)),
                               trace=_trace)
    _CACHE["last_results"] = res
    outT = np.concatenate([res.results[c]["outT"] for c in range(N_CORES)],
                          axis=1)                      # (128, 16384)
    return np.ascontiguousarray(outT.T).astype(np.float32)


# revision 12
# speedup vs baseline: 1.2197x; 1.1904x over previous
"""MFA per-component log-likelihood kernel for 8x TRN2 NeuronCores.

Math: out[n,k] = base[k] + sum_m (x_n . g_km)^2 + x_n . Wx_k + (x_n^2) . Wxx_k
with g/Wx/base from the Woodbury factorization (host-side, tiny).

Device strategy (per core, N_SHARD=2048 rows, output TRANSPOSED [K, n]):
  - Weights-stationary fp8 (e4m3) DoubleRow matmuls for the factor ("quad")
    columns: stationary = Gw block [128d, 2, 128cols], moving = x_fp8
    [128d, 2, 512n] -> psum y-block [128cols, 512n], 256-deep contraction
    per streamed column (2x fp32 MAC rate).
  - ScalarE squares psum with scale 1/64 into fp8 "sq" pairs.
  - Group-of-16 reduction on the PE: fp8 DoubleRow matmul against a
    constant 0/4 block-indicator matrix (S2), accumulating quad directly
    into the per-n-block accumulator psum bank [128k, 512n].
  - Linear term x.Wx: fp8 DoubleRow, stationary Wx block, same moving x,
    accumulated into the same acc bank.
  - x^2 . Wxx: fp16 matmuls (host-computed xsq fp16) into the same bank,
    deferred so the xsq DMA is never on the critical path.
  - DVE adds base (per-partition scalar) while copying acc psum -> SBUF,
    DMA out.  Host transposes [K,N] -> [N,K].

Sharding: rows N=16384 split across 8 cores; params replicated.
"""

import math

import numpy as np

K, D_FEAT, L_FAC, N = 128, 1024, 16, 16384
N_CORES = 8
N_SHARD = N // N_CORES            # 2048 rows per core
NB = N_SHARD // 512               # 4 moving blocks of 512 rows
J2 = 4                            # DoubleRow contraction chunks (256 each)
JJ = 8                            # psb fp16 chunks
CB = 16                           # 128-col blocks of factor columns
NPAIR = CB // 2                   # S2 pair matmuls per n-block
GCOLS = K * L_FAC                 # 2048 factor columns
WCOLS = GCOLS + K                 # 2176 = [Gw | Wx]
SG = 32.0                         # Gw fp8 scale
SQ_SCALE = 1.0 / 64.0             # scalar: sq = (psum/64)^2 = y^2/4
S2_VAL = 4.0                      # un-scales sq in the group-sum matmul

_CACHE = {}


def _get_nc():
    if "nc" in _CACHE:
        return _CACHE["nc"]

    import concourse.bass as bass
    import concourse.tile as tile
    from concourse import bacc, mybir

    f32 = mybir.dt.float32
    f16 = mybir.dt.float16
    f8 = mybir.dt.float8e4
    DR = mybir.MatmulPerfMode.DoubleRow
    nc = bacc.Bacc("TRN2", target_bir_lowering=False, debug=False,
                   num_devices=N_CORES)

    xq = nc.dram_tensor("xq", [128, J2, 2, N_SHARD], f8, kind="ExternalInput").ap()
    xsqh = nc.dram_tensor("xsqh", [128, JJ, N_SHARD], f16, kind="ExternalInput").ap()
    wq = nc.dram_tensor("wq", [128, J2, 2, WCOLS], f8, kind="ExternalInput").ap()
    wxxh = nc.dram_tensor("wxxh", [128, JJ, K], f16, kind="ExternalInput").ap()
    s2 = nc.dram_tensor("s2", [128, NPAIR, 2, K], f8, kind="ExternalInput").ap()
    bs = nc.dram_tensor("bs", [128, 1], f32, kind="ExternalInput").ap()
    outT = nc.dram_tensor("outT", [128, N_SHARD], f32, kind="ExternalOutput").ap()

    WPIECES = [(0, 512), (512, 1024), (1024, 1536), (1536, WCOLS)]

    with tile.TileContext(nc) as tc:
        with (
            tc.tile_pool(name="singles", bufs=1) as singles,
            tc.tile_pool(name="sqpool", bufs=3) as sqpool,
            tc.tile_pool(name="upool", bufs=2) as upool,
            tc.tile_pool(name="qp", bufs=4, space="PSUM") as qp,
            tc.tile_pool(name="accp", bufs=4, space="PSUM") as accp,
        ):
            wq_s = singles.tile([128, J2, 2, WCOLS], f8, tag="wq")
            xq_s = singles.tile([128, J2, 2, N_SHARD], f8, tag="xq")
            xs_s = singles.tile([128, JJ, N_SHARD], f16, tag="xs")
            wxx_s = singles.tile([128, JJ, K], f16, tag="wxx")
            s2_s = singles.tile([128, NPAIR, 2, K], f8, tag="s2")
            bs_s = singles.tile([128, 1], f32, tag="bs")
            dmy = singles.tile([128, 2, 512], f8, tag="dmy")

            # The memset goes on the otherwise-idle vector queue so the
            # p-state-warming dummy matmuls below can start immediately.
            nc.vector.memset(dmy, 0)

            # DMA issue order is the prefetch schedule: earliest-needed first.
            nc.sync.dma_start(out=wq_s[:, :, :, 0:512], in_=wq[:, :, :, 0:512])
            nc.sync.dma_start(out=xq_s[:, :, :, 0:512], in_=xq[:, :, :, 0:512])
            nc.sync.dma_start(out=s2_s, in_=s2)
            nc.sync.dma_start(out=wxx_s, in_=wxxh)
            nc.sync.dma_start(out=bs_s, in_=bs)
            for lo, hi in WPIECES[1:]:
                nc.sync.dma_start(out=wq_s[:, :, :, lo:hi], in_=wq[:, :, :, lo:hi])
            for nb in range(1, NB):
                nc.sync.dma_start(out=xq_s[:, :, :, nb * 512:(nb + 1) * 512],
                                  in_=xq[:, :, :, nb * 512:(nb + 1) * 512])
            for nb in range(NB):
                nc.sync.dma_start(out=xs_s[:, :, nb * 512:(nb + 1) * 512],
                                  in_=xsqh[:, :, nb * 512:(nb + 1) * 512])

            # Dummy matmuls warm the PE p-state while the first DMAs land.
            for _ in range(6):
                wrm = qp.tile([128, 512], f32, tag="q")
                nc.tensor.matmul(wrm, dmy[:, :, 0:128], dmy,
                                 start=True, stop=True, perf_mode=DR)

            # A matmul (its LDWEIGHTS) can wait on at most one not-yet-observed
            # semaphore, so touch each weight tensor with a throwaway matmul
            # before its first real use; real matmuls then wait only on their
            # data-side producer.
            for j2 in range(J2):
                warm = qp.tile([128, 512], f32, tag="q")
                nc.tensor.matmul(warm, wq_s[:, j2, :, 0:128],
                                 wq_s[:, j2, :, 0:512],
                                 start=True, stop=True, perf_mode=DR)

            accs = []
            psb_done = 0

            def emit_psb(nb):
                acc = accs[nb]
                nbs = slice(nb * 512, (nb + 1) * 512)
                for jj in range(JJ):
                    nc.tensor.matmul(acc, wxx_s[:, jj], xs_s[:, jj, nbs],
                                     start=False, stop=(jj == JJ - 1))
                u = upool.tile([128, 512], f32, tag="u")
                nc.vector.tensor_scalar_add(out=u, in0=acc, scalar1=bs_s)
                nc.gpsimd.dma_start(
                    out=outT[:, nb * 512:(nb + 1) * 512], in_=u)

            for nb in range(NB):
                acc = accp.tile([128, 512], f32, tag="acc")
                accs.append(acc)
                nbs = slice(nb * 512, (nb + 1) * 512)
                pending = None
                for pair in range(NPAIR):
                    sq_t = sqpool.tile([128, 2, 512], f8, tag="sq")
                    for r in range(2):
                        cb = 2 * pair + r
                        q = qp.tile([128, 512], f32, tag="q")
                        for j2 in range(J2):
                            nc.tensor.matmul(
                                q, wq_s[:, j2, :, cb * 128:(cb + 1) * 128],
                                xq_s[:, j2, :, nbs],
                                start=(j2 == 0), stop=(j2 == J2 - 1),
                                perf_mode=DR)
                        nc.scalar.activation(
                            sq_t[:, r, :], q,
                            mybir.ActivationFunctionType.Square,
                            scale=SQ_SCALE)
                    if nb == 0 and pair == 0:
                        # warm s2 / wxxh before their first real matmuls
                        warm2 = qp.tile([128, 512], f32, tag="q")
                        nc.tensor.matmul(warm2[:, 0:128], s2_s[:, 0], s2_s[:, 0],
                                         start=True, stop=True, perf_mode=DR)
                        warm3 = qp.tile([128, 512], f32, tag="q")
                        nc.tensor.matmul(warm3[:, 0:128], wxx_s[:, 0],
                                         wxx_s[:, 0], start=True, stop=True)
                    # defer the group-sum one pair so the square can finish
                    if pending is not None:
                        p_pair, p_sq = pending
                        nc.tensor.matmul(acc, s2_s[:, p_pair], p_sq,
                                         start=(p_pair == 0), stop=False,
                                         perf_mode=DR)
                    pending = (pair, sq_t)
                    # spread deferred psb sections out of the critical tail
                    if nb == NB - 1 and pair == 4 and psb_done < 3:
                        emit_psb(psb_done)
                        psb_done += 1
                p_pair, p_sq = pending
                nc.tensor.matmul(acc, s2_s[:, p_pair], p_sq,
                                 start=False, stop=False, perf_mode=DR)
                # linear term x . Wx (fp8, same moving x)
                for j2 in range(J2):
                    nc.tensor.matmul(acc, wq_s[:, j2, :, GCOLS:WCOLS],
                                     xq_s[:, j2, :, nbs],
                                     start=False, stop=False, perf_mode=DR)
                # deferred psb sections (wait until xsq DMA surely landed)
                if 1 <= nb < NB - 1 and psb_done < nb:
                    emit_psb(psb_done)
                    psb_done += 1

            while psb_done < NB:
                emit_psb(psb_done)
                psb_done += 1

    nc.finalize()
    _CACHE["nc"] = nc
    return nc


def _host_params(PI, MU, A, D):
    import ml_dtypes
    FP8 = ml_dtypes.float8_e4m3

    PI64 = PI.astype(np.float64)
    MU64 = MU.astype(np.float64)
    A64 = A.astype(np.float64)
    D64 = D.astype(np.float64)

    iD = D64 ** -2.0                                   # (K, d)
    iDA = iD[:, :, None] * A64                         # (K, d, l)
    Lm = np.eye(L_FAC)[None] + np.einsum("kdl,kdm->klm", A64, iDA)
    iL = np.linalg.inv(Lm)
    C = np.linalg.cholesky(iL)                         # iL = C C^T
    s = 1.0 / math.sqrt(2.0)
    G = np.einsum("kdl,klm->kdm", iDA, C) * s          # (K, d, l)
    b = np.einsum("kd,kdl->kl", MU64, iDA)             # (K, l)
    h = np.einsum("kl,klm->km", b, C) * s              # (K, l)

    Gw = G.transpose(1, 0, 2).reshape(D_FEAT, GCOLS)   # col k*16+m
    Wx = (iD * MU64).T - 2.0 * np.einsum("kdm,km->kd", G, h).T
    Wxx = -0.5 * iD.T                                  # (d, K)

    det_L = np.linalg.slogdet(Lm)[1]
    log_det_sigma = det_L - np.sum(np.log(iD), axis=1)
    c1 = np.sum(iD * MU64 * MU64, axis=1)
    hsq = np.sum(h * h, axis=1)
    base = PI64 - 0.5 * (D_FEAT * math.log(2.0 * math.pi)
                         + log_det_sigma + c1) + hsq

    wcat = np.concatenate([Gw * SG, Wx], axis=1).astype(np.float32)  # (d, 2176)
    wq = np.ascontiguousarray(
        wcat.astype(FP8).reshape(J2, 2, 128, WCOLS).transpose(2, 0, 1, 3))

    wxxh = np.ascontiguousarray(
        Wxx.astype(np.float32).reshape(JJ, 128, K).transpose(1, 0, 2)
    ).astype(np.float16)

    s2 = np.zeros((128, NPAIR, 2, K), dtype=np.float32)
    p_idx = np.arange(128)
    for pair in range(NPAIR):
        for r in range(2):
            cb = 2 * pair + r
            s2[p_idx, pair, r, cb * 8 + p_idx // 16] = S2_VAL
    s2 = s2.astype(FP8)

    bs_np = np.ascontiguousarray(base.astype(np.float32).reshape(128, 1))
    return wq, wxxh, s2, bs_np


def kernel(x, PI, MU, A, D, _trace=False):
    from concourse.bass_utils import run_bass_kernel_spmd
    import ml_dtypes
    FP8 = ml_dtypes.float8_e4m3

    x = np.asarray(x, dtype=np.float32)
    wq, wxxh, s2, bs_np = _host_params(
        np.asarray(PI), np.asarray(MU), np.asarray(A), np.asarray(D))

    in_maps = []
    for c in range(N_CORES):
        xs = x[c * N_SHARD:(c + 1) * N_SHARD]          # (2048, 1024)
        x8t = xs.astype(FP8).T                         # (1024, 2048)
        xq = np.ascontiguousarray(
            x8t.reshape(J2, 2, 128, N_SHARD).transpose(2, 0, 1, 3))
        xsqt = (xs * xs).astype(np.float16).T          # (1024, 2048)
        xsqh = np.ascontiguousarray(
            xsqt.reshape(JJ, 128, N_SHARD).transpose(1, 0, 2))
        in_maps.append({
            "xq": xq,
            "xsqh": xsqh,
            "wq": wq,
            "wxxh": wxxh,
            "s2": s2,
            "bs": bs_np,
        })

    nc = _get_nc()
    res = run_bass_kernel_spmd(nc, in_maps, list(range(N_CORES)),
                               trace=_trace)
    _CACHE["last_results"] = res
    outT = np.concatenate([res.results[c]["outT"] for c in range(N_CORES)],
                          axis=1)                      # (128, 16384)
    return np.ascontiguousarray(outT.T).astype(np.float32)
